# revision 79
# baseline (speedup 1.0000x reference)
"""CapsuleLayer (dynamic routing) Trainium2 kernel.

x[32,2048,16] f32, W[64,2048,32,16] f32 ->
  u_hat = einsum('jidk,bik->bjid'); 3 dynamic-routing iterations
  (softmax over num_capsule j, s = sum_i c*u_hat, v = squash(s),
   logits += v.u_hat); returns v [32,64,32] f32.

Sharding: the in_caps axis i is split across 8 cores (I_loc=256/core), so
W is sharded (16.75MB f16, SBUF-resident per core) and only tiny f16
s-partials [16, 2048] cross cores (one AllReduce per routing iteration
per batch-group; one merged 32-row AllReduce for iteration 0).

Per core, u_hat (16.8M elems) is recomputed on the tensor engine each
routing iteration straight from resident f16 W (u_hat never touches
DRAM): host packs W as WP[(8i,16k) part, ic, (32d,64j) free] and x as
block-diagonal stationary tiles xbd[(8i,16k), ic, (8i',16b)]; one matmul
per (i-chunk, h) yields u_hat[(8i',16b) part, (d,j) free] with per-i
values preserved by the block-diagonal structure.

Schedule (the engines are the bottleneck, DVE first — sim busy:
DVE ~388us, ACT ~345, Pool ~331, PE ~252 in a ~500us makespan):
  - iteration 0 (uniform c=1/64) skips u_hat: s0 for BOTH batch groups
    comes from one W pass with a collapsed 32-column lhsT xsum = x/64,
    interleaved with the 16-chunk W DMA so r0 compute hides the load;
    one merged 32-row squash then fans v0 out over three DGE queues.
  - logits (kept f16, 8KB/p, buying deeper tile pools): DVE multiply
    u*v_bcast, d-tree first level on DVE, rest on GPSIMD; exp (f16 out,
    fused z-accum) on ACT; c = e/z on GPSIMD (first two chunks of each
    block on DVE so they don't queue behind the new block's trees);
    cu = u16*c_bcast on DVE (all f16 so the DVE 2x mode applies); the
    (i',b) partition contraction of cu via a static block-diag identity
    lhsT matmul accumulating all 32 i-chunks in PSUM.
  - the batch elements are independent through routing, so all four
    (r, bg) blocks form ONE flat software pipeline (phase B lags phase A
    by 2 chunks ACROSS block boundaries) and each block's
    AllReduce + squash chain is injected deep into the NEXT block's
    emission stream, where its dependencies are long met -- no
    head-of-line blocking, and every boundary is hidden under the other
    batch-group's ~100us of compute.
TimelineSim (cost model): ~500 us/core (baseline 664). Relative error
vs fp32 ref: ~6.8e-4.
"""

import numpy as np

B, I_FULL, K = 32, 2048, 16
J, D = 64, 32
N_CORES = 8
I_LOC = I_FULL // N_CORES   # 256
IC = I_LOC // 8             # 32 i-chunks of 8 i's
BG = 2                      # batch groups of 16
DJ = D * J                  # 2048
ROUTINGS = 3
EPS = 1e-7

_cache = {}


def _build_program(n_cores=N_CORES, collective=True):
    import concourse.bacc as bacc
    import concourse.bass as bass
    import concourse.tile as tile
    from concourse import mybir

    f32 = mybir.dt.float32
    f16 = mybir.dt.float16

    nc = bacc.Bacc("TRN2", target_bir_lowering=False, debug=False,
                   num_devices=n_cores)

    wp_d = nc.dram_tensor("wp", [128, IC, DJ], f16, kind="ExternalInput")
    xbd_d = nc.dram_tensor("xbd", [128, IC, BG, 128], f16,
                           kind="ExternalInput")
    ones_d = nc.dram_tensor("ones", [128, 16], f16, kind="ExternalInput")
    xsum_d = nc.dram_tensor("xsum", [128, IC, 32], f16,
                            kind="ExternalInput")
    v_out = nc.dram_tensor("v_out", [B, D, J], f32, kind="ExternalOutput")

    with tile.TileContext(nc) as tc:
        with (
            tc.tile_pool(name="res", bufs=1) as res,
            tc.tile_pool(name="work", bufs=2) as work,
            tc.tile_pool(name="small", bufs=4) as small,
            tc.tile_pool(name="sq", bufs=1) as sq,
            tc.tile_pool(name="pu", bufs=2, space="PSUM") as pu,
            tc.tile_pool(name="ps", bufs=1, space="PSUM") as ps,
            tc.tile_pool(name="dram", bufs=1, space="DRAM") as dram,
        ):
            # ---------- DRAM scratch ----------
            s_part = dram.tile([B, DJ], f16, name="s_part")
            s_fulls = [[dram.tile([16, D, J], f16, name=f"s_full{r}_{bg}",
                                  addr_space="Shared")
                        for bg in range(BG)]
                       for r in range(1, ROUTINGS)]
            s_full0 = dram.tile([32, D, J], f16, name="s_full0",
                                addr_space="Shared")

            # ---------- resident SBUF ----------
            ones = res.tile([128, 16], f16, name="ones_sb")
            nc.sync.dma_start(out=ones, in_=ones_d.ap())
            # xsum shares the u16 slot pool: only live during iteration 0
            xsum = work.tile([128, IC, 32], f16, name="xsum_sb", tag="u16",
                             bufs=5)
            nc.sync.dma_start(out=xsum, in_=xsum_d.ap())
            wp = res.tile([128, IC, DJ], f16, name="wp_sb")        # 128KB/p
            # fine-grained chunk DMAs so iteration-0 matmuls run inside
            # the W load window
            NCH = 32
            icch = IC // NCH
            for ch in range(NCH):
                nc.sync.dma_start(
                    out=wp[:, ch * icch:(ch + 1) * icch, :],
                    in_=wp_d.ap()[:, ch * icch:(ch + 1) * icch, :])

            L = res.tile([128, BG, IC, J], f16, name="L_sb")       # 8KB/p
            vexp = [res.tile([128, D, J], f16, name=f"vexp{bg}_sb")
                    for bg in range(BG)]

            def emit_collective(r, bg):
                bgsl = slice(bg * 16, (bg + 1) * 16)
                if collective:
                    nc.gpsimd.collective_compute(
                        "AllReduce", mybir.AluOpType.add,
                        replica_groups=[list(range(n_cores))],
                        ins=[s_part[bgsl, :].opt()],
                        outs=[s_fulls[r - 1][bg][:, :, :].opt()])
                else:
                    nc.sync.dma_start(
                        out=s_fulls[r - 1][bg][:, :, :],
                        in_=s_part[bgsl, :].rearrange(
                            "b (d j) -> b d j", d=D))

            def emit_squash(r, bg):
                big_mul = nc.vector.tensor_mul
                bgsl = slice(bg * 16, (bg + 1) * 16)
                s_sb = sq.tile([16, D, J], f16, name="s_sb", tag="sev")
                nc.sync.dma_start(out=s_sb, in_=s_fulls[r - 1][bg][:, :, :])
                s2 = sq.tile([16, D, J], f16, name="s2")
                big_mul(out=s2, in0=s_sb, in1=s_sb)
                w = D
                while w > 1:
                    hw = w // 2
                    nc.vector.tensor_add(out=s2[:, 0:hw, :],
                                         in0=s2[:, 0:hw, :],
                                         in1=s2[:, hw:w, :])
                    w = hw
                n_t = sq.tile([16, 1, J], f32, name="n_t")       # ||s||^2
                nc.vector.tensor_copy(out=n_t, in_=s2[:, 0:1, :])
                eps_t = sq.tile([16, 1], f32, name="eps_t")
                nc.vector.memset(eps_t, EPS)
                sqr = sq.tile([16, 1, J], f32, name="sqr")       # sqrt(n+eps)
                nc.scalar.activation(out=sqr, in_=n_t,
                                     func=mybir.ActivationFunctionType.Sqrt,
                                     bias=eps_t[:], scale=1.0)
                # dummy exp: forces the exp-table reload HERE (hidden in
                # the squash window) instead of before the next block's
                # first real exp on its critical refill path
                edum = sq.tile([16, 1], f32, name="edum")
                nc.scalar.activation(out=edum, in_=eps_t,
                                     func=mybir.ActivationFunctionType.Exp)
                onep = sq.tile([16, 1, J], f32, name="onep")
                nc.scalar.add(out=onep, in_=n_t, add=1.0)
                nc.vector.tensor_mul(out=onep, in0=onep, in1=sqr)
                rec = sq.tile([16, 1, J], f32, name="rec")
                nc.vector.reciprocal(out=rec, in_=onep)
                scl = sq.tile([16, 1, J], f16, name="scl")       # n/(1+n)/sq
                nc.vector.tensor_mul(out=scl, in0=n_t, in1=rec)
                scl_ap = scl[:]
                scl_b = bass.AP(tensor=scl_ap.tensor, offset=scl_ap.offset,
                                ap=[scl_ap.ap[0], [0, D], scl_ap.ap[2]])
                if r == ROUTINGS - 1:
                    v_sb = sq.tile([16, D, J], f32, name="v_sb", tag="s2")
                    big_mul(out=v_sb, in0=s_sb, in1=scl_b)
                    nc.sync.dma_start(out=v_out.ap()[bgsl, :, :], in_=v_sb)
                else:
                    # v written straight into the i'=0 partition block of
                    # vexp, then doubled log2-style across the other 7
                    # blocks (3 chained DMAs instead of 7)
                    big_mul(out=vexp[bg][0:16, :, :],
                            in0=s_sb, in1=scl_b)
                    for rep in range(1, 8):
                        nc.sync.dma_start(
                            out=vexp[bg][rep * 16:(rep + 1) * 16, :, :],
                            in_=vexp[bg][0:16, :, :])

            class Block:
                """One (r, bg) routing pass: 32 i-chunk pipeline steps."""

                def __init__(self, r, bg):
                    self.r, self.bg = r, bg
                    self.s_ps = None
                    self.xbd_ch = {}
                    self.stash = {}

                def emit_xbd(self):
                    for q in range(4):
                        xq = work.tile([128, IC // 4, 128], f16,
                                       name="xbd", bufs=4)
                        nc.sync.dma_start(
                            out=xq,
                            in_=xbd_d.ap()[:, q * (IC // 4):
                                           (q + 1) * (IC // 4), self.bg, :])
                        self.xbd_ch[q] = xq

                def phase_a(self, ic):
                    r, bg = self.r, self.bg
                    u16 = work.tile([128, D, J], f16, name="u16", bufs=5)
                    for h in range(2):
                        u_ps = pu.tile([128, DJ // 2], f32, name="u_ps")
                        for q in range(2):
                            nc.tensor.matmul(
                                out=u_ps[:, q * 512:(q + 1) * 512],
                                lhsT=self.xbd_ch[ic // (IC // 4)][
                                    :, ic % (IC // 4), :],
                                rhs=wp[:, ic, h * (DJ // 2) + q * 512:
                                       h * (DJ // 2) + (q + 1) * 512],
                                start=True, stop=True)
                        nc.scalar.copy(
                            out=u16[:, h * 16:(h + 1) * 16, :].rearrange(
                                "p a b -> p (a b)"),
                            in_=u_ps)
                    prod = work.tile([128, D, J], f16, name="prod",
                                     bufs=4)
                    nc.vector.tensor_mul(out=prod, in0=u16, in1=vexp[bg])
                    nc.vector.tensor_add(out=prod[:, 0:16, :],
                                         in0=prod[:, 0:16, :],
                                         in1=prod[:, 16:32, :])
                    # tail of the d-tree + L update on the (idle) GPSIMD
                    nc.gpsimd.tensor_add(out=prod[:, 0:8, :],
                                         in0=prod[:, 0:8, :],
                                         in1=prod[:, 8:16, :])
                    nc.gpsimd.tensor_add(out=prod[:, 0:4, :],
                                         in0=prod[:, 0:4, :],
                                         in1=prod[:, 4:8, :])
                    nc.gpsimd.tensor_add(out=prod[:, 0:2, :],
                                         in0=prod[:, 0:2, :],
                                         in1=prod[:, 2:4, :])
                    if r == 1:
                        nc.gpsimd.tensor_add(out=L[:, bg, ic, :],
                                             in0=prod[:, 0, :],
                                             in1=prod[:, 1, :])
                    else:
                        ltmp = small.tile([128, J], f32, name="ltmp")
                        nc.gpsimd.tensor_add(out=ltmp, in0=prod[:, 0, :],
                                             in1=prod[:, 1, :])
                        nc.gpsimd.tensor_add(out=L[:, bg, ic, :],
                                             in0=L[:, bg, ic, :],
                                             in1=ltmp)
                    # exp on ACT: e = exp(L) (f16 for DVE 2x), z = sum_j e
                    e16 = small.tile([128, J], f16, name="e16")
                    z_t = small.tile([128, 1], f32, name="z_t")
                    nc.scalar.activation(
                        out=e16, in_=L[:, bg, ic, :],
                        func=mybir.ActivationFunctionType.Exp,
                        accum_out=z_t)
                    self.stash[ic] = (u16, e16, z_t)

                def phase_b(self, ic):
                    if self.s_ps is None:
                        self.s_ps = ps.tile([16, DJ], f32, name="s_ps",
                                            tag="s")
                    u16, e16, z_t = self.stash.pop(ic)
                    zi = small.tile([128, 1], f32, name="zi")
                    nc.vector.reciprocal(out=zi, in_=z_t)
                    # c = e/z on GPSIMD (tiny), keeping the big cu multiply
                    # on DVE as TensorTensor (2x f16 mode); the first two
                    # chunks of a block use DVE so they don't queue behind
                    # the new block's trees on the in-order GPSIMD
                    c16 = small.tile([128, J], f16, name="c16")
                    ceng = nc.vector if ic < 2 else nc.gpsimd
                    ceng.tensor_scalar_mul(out=c16, in0=e16,
                                           scalar1=zi)
                    cu = work.tile([128, D, J], f16, name="cu", tag="prod",
                                   bufs=4)
                    c_ap = c16[:]
                    c_b = bass.AP(tensor=c_ap.tensor, offset=c_ap.offset,
                                  ap=[c_ap.ap[0], [0, D], c_ap.ap[1]])
                    nc.vector.tensor_mul(out=cu, in0=u16, in1=c_b)
                    rflat = cu[:].rearrange("p a b -> p (a b)")
                    for q in range(4):
                        nc.tensor.matmul(
                            out=self.s_ps[:, q * 512:(q + 1) * 512],
                            lhsT=ones,
                            rhs=rflat[:, q * 512:(q + 1) * 512],
                            start=(ic == 0), stop=(ic == IC - 1),
                            skip_group_check=True)

                def emit_tail(self):
                    # PSUM is not DMA-readable: bounce via SBUF
                    bg = self.bg
                    s_ev = sq.tile([16, DJ], f16, name="s_ev", tag="sev")
                    nc.scalar.copy(out=s_ev, in_=self.s_ps)
                    nc.sync.dma_start(
                        out=s_part[bg * 16:(bg + 1) * 16, :], in_=s_ev)

            # ---------- iteration 0: both bgs in one W pass ----------
            s_ps0 = ps.tile([32, DJ], f32, name="s_ps0", tag="s")
            for ic in range(IC):
                for q in range(4):
                    nc.tensor.matmul(
                        out=s_ps0[:, q * 512:(q + 1) * 512],
                        lhsT=xsum[:, ic, :],
                        rhs=wp[:, ic, q * 512:(q + 1) * 512],
                        start=(ic == 0), stop=(ic == IC - 1),
                        skip_group_check=True)
            s_ev0 = sq.tile([32, DJ], f16, name="s_ev0", tag="sev")
            nc.scalar.copy(out=s_ev0, in_=s_ps0)
            nc.sync.dma_start(out=s_part[:, :], in_=s_ev0)
            # r0 has both bgs' s ready at once: ONE 32-row collective and
            # ONE squash chain produce v0 for both (this chain gates r1
            # start; engines are idle during the W load anyway).
            if collective:
                nc.gpsimd.collective_compute(
                    "AllReduce", mybir.AluOpType.add,
                    replica_groups=[list(range(n_cores))],
                    ins=[s_part[:, :].opt()],
                    outs=[s_full0[:, :, :].opt()])
            else:
                nc.sync.dma_start(
                    out=s_full0[:, :, :],
                    in_=s_part[:, :].rearrange("b (d j) -> b d j", d=D))
            s_sb0 = sq.tile([32, D, J], f16, name="s_sb0", tag="sev")
            nc.sync.dma_start(out=s_sb0, in_=s_full0[:, :, :])
            s2_0 = sq.tile([32, D, J], f16, name="s2_0", tag="s2")
            nc.vector.tensor_mul(out=s2_0, in0=s_sb0, in1=s_sb0)
            w = D
            while w > 1:
                hw = w // 2
                nc.vector.tensor_add(out=s2_0[:, 0:hw, :],
                                     in0=s2_0[:, 0:hw, :],
                                     in1=s2_0[:, hw:w, :])
                w = hw
            n_t0 = sq.tile([32, 1, J], f32, name="n_t0", tag="n_t")
            nc.vector.tensor_copy(out=n_t0, in_=s2_0[:, 0:1, :])
            eps0 = sq.tile([32, 1], f32, name="eps0", tag="eps_t")
            nc.vector.memset(eps0, EPS)
            sqr0 = sq.tile([32, 1, J], f32, name="sqr0", tag="sqr")
            nc.scalar.activation(out=sqr0, in_=n_t0,
                                 func=mybir.ActivationFunctionType.Sqrt,
                                 bias=eps0[:], scale=1.0)
            edum0 = sq.tile([32, 1], f32, name="edum0", tag="edum")
            nc.scalar.activation(out=edum0, in_=eps0,
                                 func=mybir.ActivationFunctionType.Exp)
            onep0 = sq.tile([32, 1, J], f32, name="onep0", tag="onep")
            nc.scalar.add(out=onep0, in_=n_t0, add=1.0)
            nc.vector.tensor_mul(out=onep0, in0=onep0, in1=sqr0)
            rec0 = sq.tile([32, 1, J], f32, name="rec0", tag="rec")
            nc.vector.reciprocal(out=rec0, in_=onep0)
            scl0 = sq.tile([32, 1, J], f16, name="scl0", tag="scl")
            nc.vector.tensor_mul(out=scl0, in0=n_t0, in1=rec0)
            scl0_ap = scl0[:]
            scl0_b = bass.AP(tensor=scl0_ap.tensor, offset=scl0_ap.offset,
                             ap=[scl0_ap.ap[0], [0, D], scl0_ap.ap[2]])
            # bg0's v: direct mul into vexp[0] (base partition 0 is
            # legal); bg1's rows start at partition 16 so they go through
            # a full-width temp + DMA (partition slices unrestricted)
            v32 = sq.tile([32, D, J], f16, name="v32", tag="s2")
            nc.vector.tensor_mul(out=v32, in0=s_sb0, in1=scl0_b)
            # flat fan-out of v0 over four idle DGE queues: every block of
            # vexp0 lands ~one DMA-hop after the mul (vexp0 gates r1;
            # vexp1 trails behind on the same queues, needed a block later)
            engs = [nc.sync, nc.scalar, nc.gpsimd]
            for rep in range(8):
                engs[rep % 3].dma_start(
                    out=vexp[0][rep * 16:(rep + 1) * 16, :, :],
                    in_=v32[0:16, :, :])
            for rep in range(8):
                engs[rep % 3].dma_start(
                    out=vexp[1][rep * 16:(rep + 1) * 16, :, :],
                    in_=v32[16:32, :, :])

            # ---------- iterations 1..2: one flat pipeline ----------
            # All four (r, bg) blocks form a single software-pipelined
            # emission stream so each block's head fills the engine gaps
            # left by the previous block's drain, and squash chains are
            # injected deep into the NEXT block's stream (their collective
            # dependencies are long met there — no head-of-line blocking).
            blocks = [Block(r, bg) for r in range(1, ROUTINGS)
                      for bg in range(BG)]
            blocks[0].emit_xbd()
            LAG = 2
            flat = [(n, ic) for n in range(len(blocks))
                    for ic in range(IC)]
            # injections keyed by phase-A flat position
            inject = {}
            for n in range(len(blocks)):
                base = n * IC
                if n + 1 < len(blocks):
                    inject.setdefault(base + 20, []).append(
                        blocks[n + 1].emit_xbd)
                prev = blocks[n - 1] if n > 0 else None
                if prev is not None:
                    inject.setdefault(base + 5, []).append(prev.emit_tail)
                    inject.setdefault(
                        base + 7, []).append(
                        lambda r=prev.r, bg=prev.bg: emit_collective(r, bg))
                    inject.setdefault(
                        base + 15, []).append(
                        lambda r=prev.r, bg=prev.bg: emit_squash(r, bg))
            for t in range(len(flat) + LAG):
                if t < len(flat):
                    n, ic = flat[t]
                    blocks[n].phase_a(ic)
                    for thunk in inject.get(t, []):
                        thunk()
                if t >= LAG:
                    n, ic = flat[t - LAG]
                    blocks[n].phase_b(ic)
            last = blocks[-1]
            last.emit_tail()
            emit_collective(last.r, last.bg)
            emit_squash(last.r, last.bg)

    nc.compile()
    return nc


def _pack_inputs(x, W):
    """Host-side packing of per-core kernel inputs."""
    in_maps = []
    base = np.zeros((128, 16), np.float32)
    for i in range(8):
        base[i * 16:(i + 1) * 16] = np.eye(16)
    ones = base.astype(np.float16)                      # [128, 16]
    for c in range(N_CORES):
        sl = slice(c * I_LOC, (c + 1) * I_LOC)
        Wc = W[:, sl]                                   # [J, 256, D, K]
        wp = Wc.reshape(J, IC, 8, D, K).transpose(2, 4, 1, 3, 0)
        # wp: [i, k, ic, d, j] -> [(i k)=128, IC, DJ]
        wp = np.ascontiguousarray(wp).reshape(128, IC, DJ).astype(np.float16)
        xc = x[:, sl]                                   # [B, 256, K]
        xcol = xc.reshape(BG, 16, IC, 8, K).transpose(3, 4, 2, 0, 1)
        # xcol: [i, k, ic, bg, b]
        xbd = np.zeros((8, K, IC, BG, 8, 16), np.float32)
        for i in range(8):
            xbd[i, :, :, :, i, :] = xcol[i]
        xbd = xbd.reshape(128, IC, BG, 128)
        xsum = (xcol / 64.0).reshape(128, IC, 32)       # cols = (bg, b)
        in_maps.append({"wp": wp, "xbd": xbd.astype(np.float16),
                        "xsum": xsum.astype(np.float16),
                        "ones": ones})
    return in_maps


def kernel(x, W, trace=False, trace_kwargs=None):
    from concourse import bass_utils
    if "nc" not in _cache:
        _cache["nc"] = _build_program()
    nc = _cache["nc"]
    in_maps = _pack_inputs(np.asarray(x, np.float32),
                           np.asarray(W, np.float32))
    res = bass_utils.run_bass_kernel_spmd(
        nc, in_maps, core_ids=list(range(N_CORES)), trace=trace,
        **(trace_kwargs or {}))
    if trace:
        _cache["last_results"] = res
    v = res.results[0]["v_out"]          # [B, D, J]
    return np.ascontiguousarray(v.transpose(0, 2, 1))  # [B, J, D]
